# Initial kernel scaffold
#
"""CapsuleLayer kernel for 8 Trainium2 NeuronCores (self-contained).

Strategy:
  Phase 1 (data-parallel over batch, 16 examples/core):
    - primary-capsule conv as 648 f32r matmuls (81 kernel shifts x 2 cin tiles
      x 2 outch tiles x 2 batch halves), PSUM-accumulated. f32r (fp32 w/
      11-bit mantissa, host-rounded) streams at 1 cyc/row.
    - bias add + squash over the 8 capsules (capsule index lives on the
      partition axis, summed via a constant selection matmul).
  Reshard (AllToAll): x [B,1152,8] moves from batch-sharded to route-sharded
      (each core keeps all 128 examples for its 144 routes).
  Phase 2 (route-parallel routing):
    - pred[c,b,r,o] generated by 144 K=8 matmuls (4-way row-packed, fp32).
    - 3 routing iterations; the only cross-core traffic is one fused
      AllReduce of [s_hat | Z] (= 128x170 fp32) per iteration. The
      batch-axis squash becomes core-local because every core holds the
      full batch for its route shard.
"""

import numpy as np

B = 128
NCORES = 8
BLOC = B // NCORES          # 16
CIN = 256
KCAP = 8                    # capsules (i)
OCAP = 32                   # out channels per capsule
OUTCH = KCAP * OCAP         # 256
HW_IN = 20
KH = 9
OH = 6
S = OH * OH                 # 36 spatial positions
R = OCAP * S                # 1152 routes
RLOC = R // NCORES          # 144 routes per core
OCL = OCAP // NCORES        # 4 "oc" channels per core
C = 10                      # classes
O16 = 16                    # routing output dim
NSHIFT = KH * KH            # 81
NITER = 3

PRED_DT_NAME = "bfloat16"   # "bfloat16" or "float32" for the pred tensor
GP_SPLIT = 7                # classes [0, GP_SPLIT) on vector, rest on gpsimd

_CACHE = {}


def _to_f32r(x):
    u = np.ascontiguousarray(x, dtype=np.float32).view(np.uint32)
    u = ((u.astype(np.uint64) + (1 << 11)) & 0xFFFFF000).astype(np.uint32)
    return u.view(np.float32)


def _build_program(profile=False, stop_stage=99):
    import concourse.bass as bass
    import concourse.tile as tile
    import concourse.mybir as mybir
    from concourse import bacc
    from contextlib import ExitStack

    dt = mybir.dt
    PRED_DT = getattr(dt, PRED_DT_NAME)

    nc = bacc.Bacc("TRN2", target_bir_lowering=False, debug=False,
                   num_devices=1 if profile else NCORES)

    inp_d = nc.dram_tensor("inp", [BLOC, CIN, HW_IN, HW_IN], dt.float32r,
                           kind="ExternalInput").ap()
    wconv_d = nc.dram_tensor("wconv", [NSHIFT, 2, 128, OUTCH], dt.float32r,
                             kind="ExternalInput").ap()
    bias_d = nc.dram_tensor("bias", [2, 128], dt.float32,
                            kind="ExternalInput").ap()
    wroute_d = nc.dram_tensor("wroute", [OCL, KCAP, S, C * O16], dt.float32,
                              kind="ExternalInput").ap()
    sel8_d = nc.dram_tensor("sel8", [128, 32], dt.float32,
                            kind="ExternalInput").ap()
    sel32_d = nc.dram_tensor("sel32", [32, 128], dt.float32,
                             kind="ExternalInput").ap()
    ones_d = nc.dram_tensor("ones", [128, 1], dt.float32,
                            kind="ExternalInput").ap()
    onesr_d = nc.dram_tensor("onesr", [1, 128], dt.float32,
                             kind="ExternalInput").ap()
    vout_d = nc.dram_tensor("vout", [B, C * O16], dt.float32,
                            kind="ExternalOutput").ap()

    RGRP = [list(range(NCORES))]
    PIX = BLOC * S          # 576
    NHALF = PIX // 2        # 288

    def _emit():
      with tile.TileContext(nc) as tc, ExitStack() as top:
          dram = top.enter_context(tc.tile_pool(name="dram", bufs=1, space="DRAM"))
          persist = top.enter_context(tc.tile_pool(name="persist", bufs=1))
          routing = top.enter_context(tc.tile_pool(name="routing", bufs=1))

          # chunk j -> dest core j; content [ocl, i, s, b]
          a2a_in = dram.tile([NCORES, OCL, KCAP, S, BLOC], dt.float32, tag="a2ai")
          a2a_out = dram.tile([NCORES, OCL, KCAP, S, BLOC], dt.float32, tag="a2ao")
          xdump = dram.tile([2, 128, BLOC * S], dt.float32, tag="xdump")
          xTd = dram.tile([128, S * B], dt.float32, tag="xTd")

          # ---------------- Phase 1: conv + squash ----------------
          xsb = [None, None]
          with ExitStack() as conv_scope:
              cpool = conv_scope.enter_context(tc.tile_pool(name="conv", bufs=1))
              wpool = conv_scope.enter_context(tc.tile_pool(name="wstream", bufs=4))
              cps = conv_scope.enter_context(
                  tc.tile_pool(name="cpsum", bufs=1, space="PSUM"))

              it = []
              for ct in range(2):
                  t = cpool.tile([128, BLOC * 400], dt.float32r, tag=f"in{ct}", name=f"in{ct}")
                  nc.sync.dma_start(
                      out=t[:].rearrange("ci (b f) -> ci b f", b=BLOC),
                      in_=inp_d.rearrange("b (ct ci) h w -> ct ci b (h w)", ct=2)[ct])
                  it.append(t)

              cp = [cps.tile([128, 2, 512], dt.float32, tag=f"cp{ot}", name=f"cp{ot}")
                    for ot in range(2)]

              for si in range(NSHIFT):
                  dh, dw = divmod(si, KH)
                  for ct in range(2):
                      wt = wpool.tile([128, OUTCH], dt.float32r, tag="w")
                      nc.sync.dma_start(out=wt, in_=wconv_d[si, ct])
                      for ot in range(2):
                          lhsT = wt[:, 128 * ot:128 * (ot + 1)]
                          for hh in range(2):
                              rhs = it[ct][:].rearrange(
                                  "ci (b h w) -> ci h w b", b=BLOC, h=HW_IN, w=HW_IN)[
                                  :, dh + 6 * hh:dh + 6 * hh + 6:2,
                                  dw:dw + 12:2, :]
                              nc.tensor.matmul(
                                  cp[ot][0:128, hh, 0:NHALF], lhsT, rhs,
                                  start=(si == 0 and ct == 0),
                                  stop=(si == NSHIFT - 1 and ct == 1))

              # bias add (PSUM -> SBUF)
              bt = cpool.tile([128, 2], dt.float32, tag="bias")
              nc.sync.dma_start(out=bt[:], in_=bias_d.rearrange("t p -> p t"))
              for ot in range(2):
                  x = persist.tile([128, PIX], dt.float32, tag=f"x{ot}", name=f"x{ot}")
                  for bh in range(2):
                      nc.vector.tensor_scalar_add(
                          out=x[:, NHALF * bh:NHALF * (bh + 1)],
                          in0=cp[ot][0:128, bh, 0:NHALF],
                          scalar1=bt[:, ot:ot + 1])
                  xsb[ot] = x

              # squash over capsule axis (partition groups of 32)
              sel8 = cpool.tile([128, 32], dt.float32, tag="sel8")
              sel32 = cpool.tile([32, 128], dt.float32, tag="sel32")
              nc.sync.dma_start(out=sel8, in_=sel8_d)
              nc.sync.dma_start(out=sel32, in_=sel32_d)

              sq = cpool.tile([128, PIX], dt.float32, tag="sq")
              snp = [cps.tile([32, NHALF], dt.float32, tag=f"snp{h}", name=f"snp{h}")
                     for h in range(2)]
              for ot in range(2):
                  nc.scalar.square(out=sq, in_=xsb[ot][:])
                  for h in range(2):
                      nc.tensor.matmul(
                          snp[h][0:32, 0:NHALF], sel8[:],
                          sq[:, NHALF * h:NHALF * (h + 1)],
                          start=(ot == 0), stop=(ot == 1))
              # g = sqrt(sn) / (1 + sn)
              g = cpool.tile([32, PIX], dt.float32, tag="g")
              gtmp = cpool.tile([32, PIX], dt.float32, tag="gtmp")
              for h in range(2):
                  hs = slice(NHALF * h, NHALF * (h + 1))
                  nc.scalar.sqrt(out=g[:, hs], in_=snp[h][0:32, 0:NHALF])
                  nc.vector.tensor_scalar_add(out=gtmp[:, hs],
                                              in0=snp[h][0:32, 0:NHALF],
                                              scalar1=1.0)
              nc.vector.reciprocal(out=gtmp, in_=gtmp)
              nc.vector.tensor_mul(out=g, in0=g, in1=gtmp)
              # replicate g across the 8 capsule partition groups
              grp = [cps.tile([128, NHALF], dt.float32, tag=f"grp{h}", name=f"grp{h}")
                     for h in range(2)]
              for h in range(2):
                  nc.tensor.matmul(grp[h][0:128, 0:NHALF], sel32[:],
                                   g[:, NHALF * h:NHALF * (h + 1)],
                                   start=True, stop=True)
              for ot in range(2):
                  for h in range(2):
                      hs = slice(NHALF * h, NHALF * (h + 1))
                      nc.vector.tensor_mul(out=xsb[ot][:, hs],
                                           in0=xsb[ot][:, hs],
                                           in1=grp[h][0:128, 0:NHALF])

              # stage x into the AllToAll input: chunk j holds my batch's x
              # for core j's route shard (oc = 4j + ocl), i = capsule.
              # SBUF APs must be partition-major, so bounce through DRAM and
              # do the (i2 j ocl) untangling as DRAM->DRAM copies.
              for ot in range(2):
                  nc.sync.dma_start(out=xdump[ot], in_=xsb[ot][:])
              for ot in range(2):
                  xv = xdump[ot].rearrange(
                      "(i2 j ocl) sb -> j ocl i2 sb", i2=4, j=NCORES, ocl=OCL)
                  for j in range(NCORES):
                      nc.sync.dma_start(
                          out=a2a_in[j, :, 4 * ot:4 * (ot + 1), :, :].rearrange(
                              "ocl i2 s b -> ocl i2 (s b)"),
                          in_=xv[j])

          # ---------------- Reshard ----------------
          tc.strict_bb_all_engine_barrier()
          if stop_stage <= 1:
              nc.sync.dma_start(out=vout_d, in_=xdump[0, :, 0:C * O16])
              return
          if profile:
              nc.sync.dma_start(out=a2a_out[:], in_=a2a_in[:])
          else:
              nc.gpsimd.collective_compute(
                  "AllToAll", mybir.AluOpType.bypass,
                  replica_groups=RGRP, ins=[a2a_in.opt()], outs=[a2a_out.opt()])

          # xT: [32*rg + i | q=s, b]  (rg = local oc, rows 8..31 of each strip zero)
          xT = routing.tile([128, S, B], dt.float32, tag="xT")
          wr = routing.tile([128, S, C * O16], dt.float32, tag="wr")
          nc.vector.memset(xT, 0.0)
          nc.vector.memset(wr, 0.0)
          xTdv = xTd.rearrange("(rg u) f -> rg u f", rg=4)
          for jj in range(NCORES):
              for rg in range(4):
                  nc.sync.dma_start(
                      out=xTdv[rg, 0:KCAP].rearrange(
                          "i (q jj2 b) -> i q jj2 b", q=S, jj2=NCORES)[:, :, jj, :],
                      in_=a2a_out[jj, rg])
          for rg in range(4):
              nc.sync.dma_start(
                  out=xT[32 * rg:32 * rg + KCAP, :, :],
                  in_=xTdv[rg, 0:KCAP].rearrange("i (q b) -> i q b", q=S))
              nc.sync.dma_start(
                  out=wr[32 * rg:32 * rg + KCAP, :, :],
                  in_=wroute_d[rg])

          # ---------------- pred generation ----------------
          PRED = routing.tile([128, C, O16, RLOC], PRED_DT, tag="pred")
          P4 = PRED[:].rearrange("p c o (rg q) -> p rg c o q", rg=4)
          with ExitStack() as gen_scope:
              gps = gen_scope.enter_context(
                  tc.tile_pool(name="gpsum", bufs=2, space="PSUM"))
              for q in range(S):
                  pp = gps.tile([128, 4, 512], dt.float32, tag="pp")
                  for rg in range(4):
                      nc.tensor.matmul(pp[0:128, rg, 0:C * O16],
                                       xT[32 * rg:32 * rg + KCAP, q, :],
                                       wr[32 * rg:32 * rg + KCAP, q, :],
                                       start=True, stop=True,
                                       tile_position=(32 * rg, 0))
                  src = pp[:, :, 0:C * O16].rearrange("p rg (c o) -> p rg c o", c=C)
                  dst = P4[:, :, :, :, q]
                  if q % 2 == 0:
                      nc.vector.tensor_copy(out=dst, in_=src)
                  else:
                      nc.scalar.copy(out=dst, in_=src)

          if stop_stage <= 2:
              nc.sync.dma_start(out=vout_d.rearrange('b co -> co b')[0:128, 0:128], in_=xT[:, 0, :])
              return

          # ---------------- routing iterations ----------------
          tc.strict_bb_all_engine_barrier()
          logits = routing.tile([128, C, RLOC], dt.float32, tag="logits")
          nc.vector.memset(logits, 0.0)
          e_t = routing.tile([128, C, RLOC], PRED_DT, tag="e")
          spool = top.enter_context(tc.tile_pool(name="scratch", bufs=2))
          arpack = routing.tile([128, C * O16 + C], dt.float32, tag="arpack")
          s_sb = routing.tile([128, C * O16 + C], dt.float32, tag="s_sb")
          sqs = routing.tile([128, C * O16], dt.float32, tag="sqs")
          gt1 = routing.tile([1, C * O16], dt.float32, tag="gt1")
          gt2 = routing.tile([1, C * O16], dt.float32, tag="gt2")
          vsb = routing.tile([128, C * O16], dt.float32, tag="vsb")
          vb16 = routing.tile([128, C, O16], PRED_DT, tag="vb16")
          onesb = routing.tile([128, 1], dt.float32, tag="onesb")
          onesr = routing.tile([1, 128], dt.float32, tag="onesr")
          rz = routing.tile([128, C], dt.float32, tag="rz")
          nc.sync.dma_start(out=onesb, in_=ones_d)
          nc.sync.dma_start(out=onesr, in_=onesr_d)

          def eng(c):
              return nc.vector if c < GP_SPLIT else nc.gpsimd

          with ExitStack() as it_scope:
              ips = it_scope.enter_context(
                  tc.tile_pool(name="ipsum", bufs=1, space="PSUM"))
              s0p = ips.tile([128, 512], dt.float32, tag="s0p")
              snb = ips.tile([1, C * O16], dt.float32, tag="snb")
              gbp = ips.tile([128, C * O16], dt.float32, tag="gbp")

              for t in range(NITER):
                  ZOFF = C * O16
                  if t == 0:
                      # s0 = sum_r pred (uniform routing weights), exact in fp32
                      for q in range(S):
                          nc.tensor.matmul(s0p[0:128, 0:C * O16],
                                           xT[:, q, :], wr[:, q, :],
                                           start=(q == 0), stop=(q == S - 1))
                      nc.scalar.mul(out=arpack[:, 0:ZOFF],
                                    in_=s0p[0:128, 0:C * O16], mul=1.0 / R)
                      nc.vector.memset(arpack[:, ZOFF:ZOFF + C], 1.0 / NCORES)
                  else:
                      # e = exp(logits), Z = sum_r e, s_hat = sum_r e * pred
                      nc.scalar.activation(out=e_t, in_=logits[:],
                                           func=mybir.ActivationFunctionType.Exp)
                      nc.vector.tensor_reduce(
                          out=arpack[:, ZOFF:ZOFF + C].rearrange("p (c u) -> p c u", c=C),
                          in_=e_t[:], axis=mybir.AxisListType.X,
                          op=mybir.AluOpType.add)
                      for c in range(C):
                          sfx = "v" if c < GP_SPLIT else "g"
                          scr = spool.tile([128, O16, RLOC], PRED_DT,
                                           tag=f"scr{sfx}", name=f"scr_{t}_{c}")
                          ev = e_t[:, c, :].unsqueeze(1).broadcast_to((128, O16, RLOC))
                          eng(c).tensor_mul(out=scr[:], in0=PRED[:, c, :, :], in1=ev)
                          nc.vector.tensor_reduce(
                              out=arpack[:, c * O16:(c + 1) * O16].rearrange(
                                  "p (o u) -> p o u", o=O16),
                              in_=scr[:], axis=mybir.AxisListType.X,
                              op=mybir.AluOpType.add)

                  ar_in = dram.tile([128, ZOFF + C], dt.float32, tag="arin")
                  ar_out = dram.tile([128, ZOFF + C], dt.float32, tag="arout")
                  nc.sync.dma_start(out=ar_in[:], in_=arpack[:])
                  if profile:
                      nc.sync.dma_start(out=ar_out[:], in_=ar_in[:])
                  else:
                      nc.gpsimd.collective_compute(
                          "AllReduce", mybir.AluOpType.add, replica_groups=RGRP,
                          ins=[ar_in.opt()], outs=[ar_out.opt()])
                  nc.sync.dma_start(out=s_sb, in_=ar_out[:])

                  # s = s_hat / Z
                  nc.vector.reciprocal(out=rz, in_=s_sb[:, ZOFF:ZOFF + C])
                  sv = s_sb[:, 0:ZOFF].rearrange("p (c o) -> p c o", c=C)
                  nc.vector.tensor_mul(
                      out=sv, in0=sv,
                      in1=rz[:].unsqueeze(2).broadcast_to((128, C, O16)))

                  # v = squash(s) over the (full, core-local) batch axis
                  nc.scalar.square(out=sqs, in_=s_sb[:, 0:ZOFF])
                  nc.tensor.matmul(snb[0:1, 0:ZOFF], onesb[:], sqs[:],
                                   start=True, stop=True)
                  nc.scalar.sqrt(out=gt1, in_=snb[0:1, 0:ZOFF])
                  nc.vector.tensor_scalar_add(out=gt2, in0=snb[0:1, 0:ZOFF],
                                              scalar1=1.0)
                  nc.vector.reciprocal(out=gt2, in_=gt2)
                  nc.vector.tensor_mul(out=gt1, in0=gt1, in1=gt2)
                  nc.tensor.matmul(gbp[0:128, 0:ZOFF], onesr[0:1, :], gt1[0:1, :],
                                   start=True, stop=True)
                  nc.vector.tensor_mul(out=vsb, in0=s_sb[:, 0:ZOFF],
                                       in1=gbp[0:128, 0:ZOFF])

                  if t < NITER - 1:
                      # logits += sum_o pred * v
                      nc.vector.tensor_copy(
                          out=vb16, in_=vsb[:].rearrange("p (c o) -> p c o", c=C))
                      for c in range(C):
                          sfx = "v" if c < GP_SPLIT else "g"
                          en = eng(c)
                          scr = spool.tile([128, O16, RLOC], PRED_DT,
                                           tag=f"scr{sfx}", name=f"tsc_{t}_{c}")
                          scr2 = spool.tile([128, 8, RLOC], PRED_DT,
                                            tag=f"scr2{sfx}", name=f"ts2_{t}_{c}")
                          scr3 = spool.tile([128, 4, RLOC], PRED_DT,
                                            tag=f"scr3{sfx}", name=f"ts3_{t}_{c}")
                          scr4 = spool.tile([128, 2, RLOC], PRED_DT,
                                            tag=f"scr4{sfx}", name=f"ts4_{t}_{c}")
                          scr5 = spool.tile([128, 1, RLOC], dt.float32,
                                            tag=f"scr5{sfx}", name=f"ts5_{t}_{c}")
                          vv = vb16[:, c, :].unsqueeze(2).broadcast_to(
                              (128, O16, RLOC))
                          en.tensor_mul(out=scr[:], in0=PRED[:, c, :, :], in1=vv)
                          en.tensor_add(out=scr2[:], in0=scr[:, 0:8, :],
                                        in1=scr[:, 8:16, :])
                          en.tensor_add(out=scr3[:], in0=scr2[:, 0:4, :],
                                        in1=scr2[:, 4:8, :])
                          en.tensor_add(out=scr4[:], in0=scr3[:, 0:2, :],
                                        in1=scr3[:, 2:4, :])
                          en.tensor_add(out=scr5[:], in0=scr4[:, 0:1, :],
                                        in1=scr4[:, 1:2, :])
                          en.tensor_add(
                              out=logits[:, c, :].unsqueeze(1),
                              in0=logits[:, c, :].unsqueeze(1), in1=scr5[:])

              nc.sync.dma_start(out=vout_d, in_=vsb[:])

    _emit()
    nc.compile()
    return nc


def _host_prep(inputs, conv_w, conv_b, route_weights):
    inputs = np.ascontiguousarray(inputs, dtype=np.float32)
    conv_w = np.ascontiguousarray(conv_w, dtype=np.float32)
    conv_b = np.ascontiguousarray(conv_b, dtype=np.float32)
    route_weights = np.ascontiguousarray(route_weights, dtype=np.float32)

    # conv weights -> [81, 2, 128, 256] (shift, cin_t, cin, outch=32k+oc)
    w = conv_w.reshape(OUTCH, CIN, KH, KH)          # [256 outch, 256 cin, 9, 9]
    w = w.transpose(2, 3, 1, 0).reshape(NSHIFT, 2, 128, OUTCH)
    wconv = _to_f32r(w)
    bias = conv_b.reshape(2, 128)

    # route weights per core: [ocl, i, s, c*o] with r = (4*core+ocl)*36 + s
    rw = route_weights.reshape(C, OCAP, S, KCAP, O16)   # [c, oc, s, i, o]
    wroute = []
    for core in range(NCORES):
        blk = rw[:, 4 * core:4 * core + OCL]            # [c, ocl, s, i, o]
        blk = blk.transpose(1, 3, 2, 0, 4).reshape(OCL, KCAP, S, C * O16)
        wroute.append(np.ascontiguousarray(blk))

    pidx = np.arange(128)
    sel8 = (pidx[:, None] % 32 == np.arange(32)[None, :]).astype(np.float32)
    sel32 = (np.arange(32)[:, None] == pidx[None, :] % 32).astype(np.float32)
    ones = np.ones((128, 1), dtype=np.float32)

    in_maps = []
    for core in range(NCORES):
        in_maps.append({
            "inp": _to_f32r(inputs[BLOC * core:BLOC * (core + 1)]),
            "wconv": wconv,
            "bias": bias,
            "wroute": wroute[core],
            "sel8": sel8,
            "sel32": sel32,
            "ones": ones,
            "onesr": np.ones((1, 128), dtype=np.float32),
        })
    return in_maps


def kernel(inputs, conv_w, conv_b, route_weights):
    from concourse.bass_utils import run_bass_kernel_spmd

    if "nc" not in _CACHE:
        _CACHE["nc"] = _build_program()
    nc = _CACHE["nc"]

    in_maps = _host_prep(inputs, conv_w, conv_b, route_weights)
    res = run_bass_kernel_spmd(nc, in_maps, core_ids=list(range(NCORES)))
    v = res.results[0]["vout"]                      # [128, 160]
    return np.ascontiguousarray(v.reshape(B, C, O16), dtype=np.float32)



# revision 1
# speedup vs baseline: 1.0360x; 1.0360x over previous
"""CapsuleLayer kernel for 8 Trainium2 NeuronCores (self-contained).

Strategy:
  Phase 1 (data-parallel over batch, 16 examples/core):
    - primary-capsule conv as 648 f32r matmuls (81 kernel shifts x 2 cin tiles
      x 2 outch tiles x 2 batch halves), PSUM-accumulated. f32r (fp32 w/
      11-bit mantissa, host-rounded) streams at 1 cyc/row.
    - bias add + squash over the 8 capsules (capsule index lives on the
      partition axis, summed via a constant selection matmul).
  Reshard (AllToAll): x [B,1152,8] moves from batch-sharded to route-sharded
      (each core keeps all 128 examples for its 144 routes).
  Phase 2 (route-parallel routing):
    - pred[c,b,r,o] generated by 144 K=8 matmuls (4-way row-packed, fp32).
    - 3 routing iterations; the only cross-core traffic is one fused
      AllReduce of [s_hat | Z] (= 128x170 fp32) per iteration. The
      batch-axis squash becomes core-local because every core holds the
      full batch for its route shard.
"""

import numpy as np

B = 128
NCORES = 8
BLOC = B // NCORES          # 16
CIN = 256
KCAP = 8                    # capsules (i)
OCAP = 32                   # out channels per capsule
OUTCH = KCAP * OCAP         # 256
HW_IN = 20
KH = 9
OH = 6
S = OH * OH                 # 36 spatial positions
R = OCAP * S                # 1152 routes
RLOC = R // NCORES          # 144 routes per core
OCL = OCAP // NCORES        # 4 "oc" channels per core
C = 10                      # classes
O16 = 16                    # routing output dim
NSHIFT = KH * KH            # 81
NITER = 3

PRED_DT_NAME = "bfloat16"   # "bfloat16" or "float32" for the pred tensor
GP_SPLIT = 7                # classes [0, GP_SPLIT) on vector, rest on gpsimd

_CACHE = {}


def _to_f32r(x):
    u = np.ascontiguousarray(x, dtype=np.float32).view(np.uint32)
    u = ((u.astype(np.uint64) + (1 << 11)) & 0xFFFFF000).astype(np.uint32)
    return u.view(np.float32)


def _build_program(profile=False, stop_stage=99):
    import concourse.bass as bass
    import concourse.tile as tile
    import concourse.mybir as mybir
    from concourse import bacc
    from contextlib import ExitStack

    dt = mybir.dt
    PRED_DT = getattr(dt, PRED_DT_NAME)

    nc = bacc.Bacc("TRN2", target_bir_lowering=False, debug=False,
                   num_devices=1 if profile else NCORES)

    inp_d = nc.dram_tensor("inp", [BLOC, CIN, HW_IN, HW_IN], dt.float32r,
                           kind="ExternalInput").ap()
    wconv_d = nc.dram_tensor("wconv", [NSHIFT, 2, 128, OUTCH], dt.float32r,
                             kind="ExternalInput").ap()
    bias_d = nc.dram_tensor("bias", [2, 128], dt.float32,
                            kind="ExternalInput").ap()
    wroute_d = nc.dram_tensor("wroute", [OCL, KCAP, S, C * O16], dt.float32,
                              kind="ExternalInput").ap()
    sel8_d = nc.dram_tensor("sel8", [128, 32], dt.float32,
                            kind="ExternalInput").ap()
    sel32_d = nc.dram_tensor("sel32", [32, 128], dt.float32,
                             kind="ExternalInput").ap()
    ones_d = nc.dram_tensor("ones", [128, 1], dt.float32,
                            kind="ExternalInput").ap()
    onesr_d = nc.dram_tensor("onesr", [1, 128], dt.float32,
                             kind="ExternalInput").ap()
    vout_d = nc.dram_tensor("vout", [B, C * O16], dt.float32,
                            kind="ExternalOutput").ap()

    RGRP = [list(range(NCORES))]
    PIX = BLOC * S          # 576
    NHALF = PIX // 2        # 288

    def _emit():
      with tile.TileContext(nc) as tc, ExitStack() as top:
          dram = top.enter_context(tc.tile_pool(name="dram", bufs=1, space="DRAM"))
          persist = top.enter_context(tc.tile_pool(name="persist", bufs=1))
          routing = top.enter_context(tc.tile_pool(name="routing", bufs=1))

          # chunk j -> dest core j; content [ocl, i, s, b]
          a2a_in = dram.tile([NCORES, OCL, KCAP, S, BLOC], dt.float32, tag="a2ai")
          a2a_out = dram.tile([NCORES, OCL, KCAP, S, BLOC], dt.float32, tag="a2ao")
          xdump = dram.tile([2, 128, BLOC * S], dt.float32, tag="xdump")
          xTd = dram.tile([128, S * B], dt.float32, tag="xTd")

          # ---------------- Phase 1: conv + squash ----------------
          xsb = [None, None]
          with ExitStack() as conv_scope:
              cpool = conv_scope.enter_context(tc.tile_pool(name="conv", bufs=1))
              wpool = conv_scope.enter_context(tc.tile_pool(name="wstream", bufs=4))
              cps = conv_scope.enter_context(
                  tc.tile_pool(name="cpsum", bufs=1, space="PSUM"))

              it = []
              for ct in range(2):
                  t = cpool.tile([128, BLOC * 400], dt.float32r, tag=f"in{ct}", name=f"in{ct}")
                  nc.sync.dma_start(
                      out=t[:].rearrange("ci (b f) -> ci b f", b=BLOC),
                      in_=inp_d.rearrange("b (ct ci) h w -> ct ci b (h w)", ct=2)[ct])
                  it.append(t)

              cp = [cps.tile([128, 2, 512], dt.float32, tag=f"cp{ot}", name=f"cp{ot}")
                    for ot in range(2)]

              for si in range(NSHIFT):
                  dh, dw = divmod(si, KH)
                  for ct in range(2):
                      wt = wpool.tile([128, OUTCH], dt.float32r, tag="w")
                      nc.sync.dma_start(out=wt, in_=wconv_d[si, ct])
                      for ot in range(2):
                          lhsT = wt[:, 128 * ot:128 * (ot + 1)]
                          for hh in range(2):
                              rhs = it[ct][:].rearrange(
                                  "ci (b h w) -> ci h w b", b=BLOC, h=HW_IN, w=HW_IN)[
                                  :, dh + 6 * hh:dh + 6 * hh + 6:2,
                                  dw:dw + 12:2, :]
                              nc.tensor.matmul(
                                  cp[ot][0:128, hh, 0:NHALF], lhsT, rhs,
                                  start=(si == 0 and ct == 0),
                                  stop=(si == NSHIFT - 1 and ct == 1))

              # bias add (PSUM -> SBUF)
              bt = cpool.tile([128, 2], dt.float32, tag="bias")
              nc.sync.dma_start(out=bt[:], in_=bias_d.rearrange("t p -> p t"))
              for ot in range(2):
                  x = persist.tile([128, PIX], dt.float32, tag=f"x{ot}", name=f"x{ot}")
                  for bh in range(2):
                      nc.vector.tensor_scalar_add(
                          out=x[:, NHALF * bh:NHALF * (bh + 1)],
                          in0=cp[ot][0:128, bh, 0:NHALF],
                          scalar1=bt[:, ot:ot + 1])
                  xsb[ot] = x

              # squash over capsule axis (partition groups of 32)
              sel8 = cpool.tile([128, 32], dt.float32, tag="sel8")
              sel32 = cpool.tile([32, 128], dt.float32, tag="sel32")
              nc.sync.dma_start(out=sel8, in_=sel8_d)
              nc.sync.dma_start(out=sel32, in_=sel32_d)

              sq = cpool.tile([128, PIX], dt.float32, tag="sq")
              snp = [cps.tile([32, NHALF], dt.float32, tag=f"snp{h}", name=f"snp{h}")
                     for h in range(2)]
              for ot in range(2):
                  nc.scalar.square(out=sq, in_=xsb[ot][:])
                  for h in range(2):
                      nc.tensor.matmul(
                          snp[h][0:32, 0:NHALF], sel8[:],
                          sq[:, NHALF * h:NHALF * (h + 1)],
                          start=(ot == 0), stop=(ot == 1))
              # g = sqrt(sn) / (1 + sn)
              g = cpool.tile([32, PIX], dt.float32, tag="g")
              gtmp = cpool.tile([32, PIX], dt.float32, tag="gtmp")
              for h in range(2):
                  hs = slice(NHALF * h, NHALF * (h + 1))
                  nc.scalar.sqrt(out=g[:, hs], in_=snp[h][0:32, 0:NHALF])
                  nc.vector.tensor_scalar_add(out=gtmp[:, hs],
                                              in0=snp[h][0:32, 0:NHALF],
                                              scalar1=1.0)
              nc.vector.reciprocal(out=gtmp, in_=gtmp)
              nc.vector.tensor_mul(out=g, in0=g, in1=gtmp)
              # replicate g across the 8 capsule partition groups
              grp = [cps.tile([128, NHALF], dt.float32, tag=f"grp{h}", name=f"grp{h}")
                     for h in range(2)]
              for h in range(2):
                  nc.tensor.matmul(grp[h][0:128, 0:NHALF], sel32[:],
                                   g[:, NHALF * h:NHALF * (h + 1)],
                                   start=True, stop=True)
              for ot in range(2):
                  for h in range(2):
                      hs = slice(NHALF * h, NHALF * (h + 1))
                      nc.vector.tensor_mul(out=xsb[ot][:, hs],
                                           in0=xsb[ot][:, hs],
                                           in1=grp[h][0:128, 0:NHALF])

              # stage x into the AllToAll input: chunk j holds my batch's x
              # for core j's route shard (oc = 4j + ocl), i = capsule.
              # SBUF APs must be partition-major, so bounce through DRAM and
              # do the (i2 j ocl) untangling as DRAM->DRAM copies.
              for ot in range(2):
                  nc.sync.dma_start(out=xdump[ot], in_=xsb[ot][:])
              for ot in range(2):
                  xv = xdump[ot].rearrange(
                      "(i2 j ocl) sb -> j ocl i2 sb", i2=4, j=NCORES, ocl=OCL)
                  for j in range(NCORES):
                      nc.sync.dma_start(
                          out=a2a_in[j, :, 4 * ot:4 * (ot + 1), :, :].rearrange(
                              "ocl i2 s b -> ocl i2 (s b)"),
                          in_=xv[j])

          # ---------------- Reshard ----------------
          tc.strict_bb_all_engine_barrier()
          if stop_stage <= 1:
              nc.sync.dma_start(out=vout_d, in_=xdump[0, :, 0:C * O16])
              return
          if profile:
              nc.sync.dma_start(out=a2a_out[:], in_=a2a_in[:])
          else:
              nc.gpsimd.collective_compute(
                  "AllToAll", mybir.AluOpType.bypass,
                  replica_groups=RGRP, ins=[a2a_in.opt()], outs=[a2a_out.opt()])

          # xT: [32*rg + i | q=s, b]  (rg = local oc, rows 8..31 of each strip zero)
          xT = routing.tile([128, S, B], dt.float32, tag="xT")
          wr = routing.tile([128, S, C * O16], dt.float32, tag="wr")
          nc.vector.memset(xT, 0.0)
          nc.vector.memset(wr, 0.0)
          xTdv = xTd.rearrange("(rg u) f -> rg u f", rg=4)
          for jj in range(NCORES):
              for rg in range(4):
                  nc.sync.dma_start(
                      out=xTdv[rg, 0:KCAP].rearrange(
                          "i (q jj2 b) -> i q jj2 b", q=S, jj2=NCORES)[:, :, jj, :],
                      in_=a2a_out[jj, rg])
          for rg in range(4):
              nc.sync.dma_start(
                  out=xT[32 * rg:32 * rg + KCAP, :, :],
                  in_=xTdv[rg, 0:KCAP].rearrange("i (q b) -> i q b", q=S))
              nc.sync.dma_start(
                  out=wr[32 * rg:32 * rg + KCAP, :, :],
                  in_=wroute_d[rg])

          # ---------------- pred generation ----------------
          PRED = routing.tile([128, C, O16, RLOC], PRED_DT, tag="pred")
          P4 = PRED[:].rearrange("p c o (rg q) -> p rg c o q", rg=4)
          with ExitStack() as gen_scope:
              gps = gen_scope.enter_context(
                  tc.tile_pool(name="gpsum", bufs=2, space="PSUM"))
              for q in range(S):
                  pp = gps.tile([128, 4, 512], dt.float32, tag="pp")
                  for rg in range(4):
                      nc.tensor.matmul(pp[0:128, rg, 0:C * O16],
                                       xT[32 * rg:32 * rg + KCAP, q, :],
                                       wr[32 * rg:32 * rg + KCAP, q, :],
                                       start=True, stop=True,
                                       tile_position=(32 * rg, 0))
                  src = pp[:, :, 0:C * O16].rearrange("p rg (c o) -> p rg c o", c=C)
                  dst = P4[:, :, :, :, q]
                  if q % 2 == 0:
                      nc.vector.tensor_copy(out=dst, in_=src)
                  else:
                      nc.scalar.copy(out=dst, in_=src)

          if stop_stage <= 2:
              nc.sync.dma_start(out=vout_d.rearrange('b co -> co b')[0:128, 0:128], in_=xT[:, 0, :])
              return

          # ---------------- routing iterations ----------------
          tc.strict_bb_all_engine_barrier()
          logits = routing.tile([128, C, RLOC], dt.float32, tag="logits")
          nc.vector.memset(logits, 0.0)
          e_t = routing.tile([128, C, RLOC], PRED_DT, tag="e")
          spool = top.enter_context(tc.tile_pool(name="scratch", bufs=2))
          arpack = routing.tile([128, C * O16 + C], dt.float32, tag="arpack")
          s_sb = routing.tile([128, C * O16 + C], dt.float32, tag="s_sb")
          sqs = routing.tile([128, C * O16], dt.float32, tag="sqs")
          gt1 = routing.tile([1, C * O16], dt.float32, tag="gt1")
          gt2 = routing.tile([1, C * O16], dt.float32, tag="gt2")
          vsb = routing.tile([128, C * O16], dt.float32, tag="vsb")
          vb16 = routing.tile([128, C, O16], PRED_DT, tag="vb16")
          onesb = routing.tile([128, 1], dt.float32, tag="onesb")
          onesr = routing.tile([1, 128], dt.float32, tag="onesr")
          rz = routing.tile([128, C], dt.float32, tag="rz")
          nc.sync.dma_start(out=onesb, in_=ones_d)
          nc.sync.dma_start(out=onesr, in_=onesr_d)

          def eng(c):
              return nc.vector if c < GP_SPLIT else nc.gpsimd

          with ExitStack() as it_scope:
              ips = it_scope.enter_context(
                  tc.tile_pool(name="ipsum", bufs=1, space="PSUM"))
              s0p = ips.tile([128, 512], dt.float32, tag="s0p")
              snb = ips.tile([1, C * O16], dt.float32, tag="snb")
              gbp = ips.tile([128, C * O16], dt.float32, tag="gbp")

              for t in range(NITER):
                  ZOFF = C * O16
                  if t == 0:
                      # s0 = sum_r pred (uniform routing weights), exact in fp32
                      for q in range(S):
                          nc.tensor.matmul(s0p[0:128, 0:C * O16],
                                           xT[:, q, :], wr[:, q, :],
                                           start=(q == 0), stop=(q == S - 1))
                      nc.scalar.mul(out=arpack[:, 0:ZOFF],
                                    in_=s0p[0:128, 0:C * O16], mul=1.0 / R)
                      nc.vector.memset(arpack[:, ZOFF:ZOFF + C], 1.0 / NCORES)
                  else:
                      # e = exp(logits), Z = sum_r e, s_hat = sum_r e * pred
                      nc.scalar.activation(out=e_t, in_=logits[:],
                                           func=mybir.ActivationFunctionType.Exp)
                      nc.vector.tensor_reduce(
                          out=arpack[:, ZOFF:ZOFF + C].rearrange("p (c u) -> p c u", c=C),
                          in_=e_t[:], axis=mybir.AxisListType.X,
                          op=mybir.AluOpType.add)
                      for c in range(C):
                          sfx = "v" if c < GP_SPLIT else "g"
                          scr = spool.tile([128, O16, RLOC], PRED_DT,
                                           tag=f"scr{sfx}", name=f"scr_{t}_{c}")
                          ev = e_t[:, c, :].unsqueeze(1).broadcast_to((128, O16, RLOC))
                          eng(c).tensor_mul(out=scr[:], in0=PRED[:, c, :, :], in1=ev)
                          nc.vector.tensor_reduce(
                              out=arpack[:, c * O16:(c + 1) * O16].rearrange(
                                  "p (o u) -> p o u", o=O16),
                              in_=scr[:], axis=mybir.AxisListType.X,
                              op=mybir.AluOpType.add)

                  ar_in = dram.tile([128, ZOFF + C], dt.float32, tag="arin")
                  ar_out = dram.tile([128, ZOFF + C], dt.float32, tag="arout")
                  nc.sync.dma_start(out=ar_in[:], in_=arpack[:])
                  if profile:
                      nc.sync.dma_start(out=ar_out[:], in_=ar_in[:])
                  else:
                      nc.gpsimd.collective_compute(
                          "AllReduce", mybir.AluOpType.add, replica_groups=RGRP,
                          ins=[ar_in.opt()], outs=[ar_out.opt()])
                  nc.sync.dma_start(out=s_sb, in_=ar_out[:])

                  # s = s_hat / Z
                  nc.vector.reciprocal(out=rz, in_=s_sb[:, ZOFF:ZOFF + C])
                  sv = s_sb[:, 0:ZOFF].rearrange("p (c o) -> p c o", c=C)
                  nc.vector.tensor_mul(
                      out=sv, in0=sv,
                      in1=rz[:].unsqueeze(2).broadcast_to((128, C, O16)))

                  # v = squash(s) over the (full, core-local) batch axis
                  nc.scalar.square(out=sqs, in_=s_sb[:, 0:ZOFF])
                  nc.tensor.matmul(snb[0:1, 0:ZOFF], onesb[:], sqs[:],
                                   start=True, stop=True)
                  nc.scalar.sqrt(out=gt1, in_=snb[0:1, 0:ZOFF])
                  nc.vector.tensor_scalar_add(out=gt2, in0=snb[0:1, 0:ZOFF],
                                              scalar1=1.0)
                  nc.vector.reciprocal(out=gt2, in_=gt2)
                  nc.vector.tensor_mul(out=gt1, in0=gt1, in1=gt2)
                  nc.tensor.matmul(gbp[0:128, 0:ZOFF], onesr[0:1, :], gt1[0:1, :],
                                   start=True, stop=True)
                  nc.vector.tensor_mul(out=vsb, in0=s_sb[:, 0:ZOFF],
                                       in1=gbp[0:128, 0:ZOFF])

                  if t < NITER - 1:
                      # logits += sum_o pred * v
                      nc.vector.tensor_copy(
                          out=vb16, in_=vsb[:].rearrange("p (c o) -> p c o", c=C))
                      for c in range(C):
                          sfx = "v" if c < GP_SPLIT else "g"
                          en = eng(c)
                          scr = spool.tile([128, O16, RLOC], PRED_DT,
                                           tag=f"scr{sfx}", name=f"tsc_{t}_{c}")
                          scr2 = spool.tile([128, 8, RLOC], PRED_DT,
                                            tag=f"scr2{sfx}", name=f"ts2_{t}_{c}")
                          scr3 = spool.tile([128, 4, RLOC], PRED_DT,
                                            tag=f"scr3{sfx}", name=f"ts3_{t}_{c}")
                          scr4 = spool.tile([128, 2, RLOC], PRED_DT,
                                            tag=f"scr4{sfx}", name=f"ts4_{t}_{c}")
                          scr5 = spool.tile([128, 1, RLOC], dt.float32,
                                            tag=f"scr5{sfx}", name=f"ts5_{t}_{c}")
                          vv = vb16[:, c, :].unsqueeze(2).broadcast_to(
                              (128, O16, RLOC))
                          en.tensor_mul(out=scr[:], in0=PRED[:, c, :, :], in1=vv)
                          en.tensor_add(out=scr2[:], in0=scr[:, 0:8, :],
                                        in1=scr[:, 8:16, :])
                          en.tensor_add(out=scr3[:], in0=scr2[:, 0:4, :],
                                        in1=scr2[:, 4:8, :])
                          en.tensor_add(out=scr4[:], in0=scr3[:, 0:2, :],
                                        in1=scr3[:, 2:4, :])
                          en.tensor_add(out=scr5[:], in0=scr4[:, 0:1, :],
                                        in1=scr4[:, 1:2, :])
                          en.tensor_add(
                              out=logits[:, c, :].unsqueeze(1),
                              in0=logits[:, c, :].unsqueeze(1), in1=scr5[:])

              nc.sync.dma_start(out=vout_d, in_=vsb[:])

    _emit()
    nc.compile()
    return nc


def _host_prep(inputs, conv_w, conv_b, route_weights):
    inputs = np.ascontiguousarray(inputs, dtype=np.float32)
    conv_w = np.ascontiguousarray(conv_w, dtype=np.float32)
    conv_b = np.ascontiguousarray(conv_b, dtype=np.float32)
    route_weights = np.ascontiguousarray(route_weights, dtype=np.float32)

    # conv weights -> [81, 2, 128, 256] (shift, cin_t, cin, outch=32k+oc)
    w = conv_w.reshape(OUTCH, CIN, KH, KH)          # [256 outch, 256 cin, 9, 9]
    w = w.transpose(2, 3, 1, 0).reshape(NSHIFT, 2, 128, OUTCH)
    wconv = _to_f32r(w)
    bias = conv_b.reshape(2, 128)

    # route weights per core: [ocl, i, s, c*o] with r = (4*core+ocl)*36 + s
    rw = route_weights.reshape(C, OCAP, S, KCAP, O16)   # [c, oc, s, i, o]
    wroute = []
    for core in range(NCORES):
        blk = rw[:, 4 * core:4 * core + OCL]            # [c, ocl, s, i, o]
        blk = blk.transpose(1, 3, 2, 0, 4).reshape(OCL, KCAP, S, C * O16)
        wroute.append(np.ascontiguousarray(blk))

    pidx = np.arange(128)
    sel8 = (pidx[:, None] % 32 == np.arange(32)[None, :]).astype(np.float32)
    sel32 = (np.arange(32)[:, None] == pidx[None, :] % 32).astype(np.float32)
    ones = np.ones((128, 1), dtype=np.float32)

    in_maps = []
    for core in range(NCORES):
        in_maps.append({
            "inp": _to_f32r(inputs[BLOC * core:BLOC * (core + 1)]),
            "wconv": wconv,
            "bias": bias,
            "wroute": wroute[core],
            "sel8": sel8,
            "sel32": sel32,
            "ones": ones,
            "onesr": np.ones((1, 128), dtype=np.float32),
        })
    return in_maps


def kernel(inputs, conv_w, conv_b, route_weights):
    from concourse.bass_utils import run_bass_kernel_spmd

    if "nc" not in _CACHE:
        _CACHE["nc"] = _build_program()
    nc = _CACHE["nc"]

    in_maps = _host_prep(inputs, conv_w, conv_b, route_weights)
    res = run_bass_kernel_spmd(nc, in_maps, core_ids=list(range(NCORES)))
    v = res.results[0]["vout"]                      # [128, 160]
    return np.ascontiguousarray(v.reshape(B, C, O16), dtype=np.float32)



# revision 7
# speedup vs baseline: 1.1629x; 1.1224x over previous
"""CapsuleLayer kernel for 8 Trainium2 NeuronCores (self-contained).

Strategy (v2):
  Phase 1 (data-parallel over batch, 16 examples/core):
    - primary-capsule conv as 648 f32r matmuls (81 kernel shifts x 2 cin tiles
      x 2 outch tiles x 2 batch-halves), PSUM-accumulated. PSUM free layout is
      (batch, spatial) so the resharded x is directly DMA-able.
    - bias add + squash over the 8 capsules (capsule index lives on the
      partition axis, summed via a constant selection matmul); output fp16.
  Reshard (AllToAll, fp16): x moves from batch-sharded to route-sharded with
      zero DRAM bounce copies: 2 SBUF->DRAM staging DMAs, A2A, 1 DRAM->SBUF
      load into xT [strips=(ocl,i), b, q].
  Phase 2 (route-parallel routing, fp16 elementwise):
    - pred[b, o, c, r] via 144 K=8 matmuls (4-way row-packed); copied
      PSUM->SBUF into an o-sliceable layout so routing needs no broadcasts.
    - s_hat = sum_r e*pred: 16 full-width contiguous fp16 muls (DVE 2x mode)
      + in-place binary-tree adds over r + one small fp32 tensor_reduce.
    - logits update: per-(c,o) fused scalar_tensor_tensor chains
      (out = pred*v_col + running) -- no broadcasts, no big reduces.
    - one fused AllReduce of [s_hat | Z] (128x170 fp32) per iteration.
"""

import numpy as np

B = 128
NCORES = 8
BLOC = B // NCORES          # 16
CIN = 256
KCAP = 8                    # capsules (i)
OCAP = 32                   # out channels per capsule
OUTCH = KCAP * OCAP         # 256
HW_IN = 20
KH = 9
OH = 6
S = OH * OH                 # 36 spatial positions
R = OCAP * S                # 1152 routes
RLOC = R // NCORES          # 144 routes per core
OCL = OCAP // NCORES        # 4 "oc" channels per core
C = 10                      # classes
O16 = 16                    # routing output dim
CO = C * O16                # 160
NSHIFT = KH * KH            # 81
NITER = 3
NHALF = BLOC * S // 2       # 288 (batch-half x spatial)

GP_MUL_SPLIT = 13           # o-slices [0,13) on vector, [13,16) on gpsimd
GP_DOT_SPLIT = 10           # classes [0,GP_DOT_SPLIT) on vector, rest on gpsimd
                            # (scalar_tensor_tensor only lowers on DVE)

_CACHE = {}


def _to_f32r(x):
    u = np.ascontiguousarray(x, dtype=np.float32).view(np.uint32)
    u = ((u.astype(np.uint64) + (1 << 11)) & 0xFFFFF000).astype(np.uint32)
    return u.view(np.float32)


def _build_program(profile=False):
    import concourse.bass as bass
    import concourse.tile as tile
    import concourse.mybir as mybir
    from concourse import bacc
    from contextlib import ExitStack

    dt = mybir.dt
    F16 = dt.float16
    alu = mybir.AluOpType

    nc = bacc.Bacc("TRN2", target_bir_lowering=False, debug=False,
                   num_devices=1 if profile else NCORES)

    inp_d = nc.dram_tensor("inp", [BLOC, CIN, HW_IN, HW_IN], dt.float32r,
                           kind="ExternalInput").ap()
    wconv_d = nc.dram_tensor("wconv", [NSHIFT, 2, 128, OUTCH], dt.float32r,
                             kind="ExternalInput").ap()
    bias_d = nc.dram_tensor("bias", [2, 128], dt.float32,
                            kind="ExternalInput").ap()
    wroute_d = nc.dram_tensor("wroute", [OCL, KCAP, S, CO], F16,
                              kind="ExternalInput").ap()
    sel8_d = nc.dram_tensor("sel8", [128, 32], dt.float32,
                            kind="ExternalInput").ap()
    sel32_d = nc.dram_tensor("sel32", [32, 128], dt.float32,
                             kind="ExternalInput").ap()
    ones_d = nc.dram_tensor("ones", [128, 1], dt.float32,
                            kind="ExternalInput").ap()
    onesr_d = nc.dram_tensor("onesr", [1, 128], dt.float32,
                             kind="ExternalInput").ap()
    vout_d = nc.dram_tensor("vout", [B, CO], dt.float32,
                            kind="ExternalOutput").ap()

    RGRP = [list(range(NCORES))]

    def _emit():
      with tile.TileContext(nc) as tc, ExitStack() as top:
          dram = top.enter_context(tc.tile_pool(name="dram", bufs=1, space="DRAM"))
          routing = top.enter_context(tc.tile_pool(name="routing", bufs=1))

          # A2A chunk j -> dest core j; content [ocl, i, b, s] fp16
          a2a_in = dram.tile([NCORES, OCL, KCAP, BLOC, S], F16, tag="a2ai")
          a2a_out = dram.tile([NCORES, OCL, KCAP, BLOC, S], F16, tag="a2ao")

          # ---------------- Phase 1: conv + squash ----------------
          with ExitStack() as conv_scope:
              cpool = conv_scope.enter_context(tc.tile_pool(name="conv", bufs=1))
              wpool = conv_scope.enter_context(tc.tile_pool(name="wstream", bufs=4))
              cps = conv_scope.enter_context(
                  tc.tile_pool(name="cpsum", bufs=1, space="PSUM"))

              it = []
              for ct in range(2):
                  t = cpool.tile([128, BLOC * 400], dt.float32r, tag=f"in{ct}",
                                 name=f"in{ct}")
                  nc.sync.dma_start(
                      out=t[:].rearrange("ci (b f) -> ci b f", b=BLOC),
                      in_=inp_d.rearrange("b (ct ci) h w -> ct ci b (h w)", ct=2)[ct])
                  it.append(t)

              cp = [cps.tile([128, 2, 512], dt.float32, tag=f"cp{ot}", name=f"cp{ot}")
                    for ot in range(2)]

              for si in range(NSHIFT):
                  dh, dw = divmod(si, KH)
                  for ct in range(2):
                      wt = wpool.tile([128, OUTCH], dt.float32r, tag="w")
                      nc.sync.dma_start(out=wt, in_=wconv_d[si, ct])
                      for ot in range(2):
                          lhsT = wt[:, 128 * ot:128 * (ot + 1)]
                          for bh in range(2):
                              rhs = it[ct][:].rearrange(
                                  "ci (b h w) -> ci b h w", b=BLOC, h=HW_IN)[
                                  :, 8 * bh:8 * bh + 8,
                                  dh:dh + 12:2, dw:dw + 12:2]
                              nc.tensor.matmul(
                                  cp[ot][0:128, bh, 0:NHALF], lhsT, rhs,
                                  start=(si == 0 and ct == 0),
                                  stop=(si == NSHIFT - 1 and ct == 1))

              # bias add (PSUM -> SBUF); free layout (b16, s36)
              bt = cpool.tile([128, 2], dt.float32, tag="bias")
              nc.sync.dma_start(out=bt[:], in_=bias_d.rearrange("t p -> p t"))
              xsb = []
              for ot in range(2):
                  x = cpool.tile([128, 2 * NHALF], dt.float32, tag=f"x{ot}",
                                 name=f"x{ot}")
                  for bh in range(2):
                      nc.vector.tensor_scalar_add(
                          out=x[:, NHALF * bh:NHALF * (bh + 1)],
                          in0=cp[ot][0:128, bh, 0:NHALF],
                          scalar1=bt[:, ot:ot + 1])
                  xsb.append(x)

              # squash over capsule axis (partition groups of 32)
              sel8 = cpool.tile([128, 32], dt.float32, tag="sel8")
              sel32 = cpool.tile([32, 128], dt.float32, tag="sel32")
              nc.sync.dma_start(out=sel8, in_=sel8_d)
              nc.sync.dma_start(out=sel32, in_=sel32_d)

              sq = cpool.tile([128, 2 * NHALF], dt.float32, tag="sq")
              snp = [cps.tile([32, NHALF], dt.float32, tag=f"snp{h}", name=f"snp{h}")
                     for h in range(2)]
              for ot in range(2):
                  nc.scalar.square(out=sq, in_=xsb[ot][:])
                  for h in range(2):
                      nc.tensor.matmul(
                          snp[h][0:32, 0:NHALF], sel8[:],
                          sq[:, NHALF * h:NHALF * (h + 1)],
                          start=(ot == 0), stop=(ot == 1))
              # g = sqrt(sn) / (1 + sn)
              g = cpool.tile([32, 2 * NHALF], dt.float32, tag="g")
              gtmp = cpool.tile([32, 2 * NHALF], dt.float32, tag="gtmp")
              for h in range(2):
                  hs = slice(NHALF * h, NHALF * (h + 1))
                  nc.scalar.sqrt(out=g[:, hs], in_=snp[h][0:32, 0:NHALF])
                  nc.vector.tensor_scalar_add(out=gtmp[:, hs],
                                              in0=snp[h][0:32, 0:NHALF],
                                              scalar1=1.0)
              nc.vector.reciprocal(out=gtmp, in_=gtmp)
              nc.vector.tensor_mul(out=g, in0=g, in1=gtmp)
              # replicate g across the 8 capsule partition groups
              grp = [cps.tile([128, NHALF], dt.float32, tag=f"grp{h}", name=f"grp{h}")
                     for h in range(2)]
              for h in range(2):
                  nc.tensor.matmul(grp[h][0:128, 0:NHALF], sel32[:],
                                   g[:, NHALF * h:NHALF * (h + 1)],
                                   start=True, stop=True)
              # x * g, cast fp16
              xsb2 = []
              for ot in range(2):
                  x2 = cpool.tile([128, 2 * NHALF], F16, tag=f"x2{ot}",
                                  name=f"x2{ot}")
                  for h in range(2):
                      hs = slice(NHALF * h, NHALF * (h + 1))
                      nc.vector.tensor_mul(out=x2[:, hs], in0=xsb[ot][:, hs],
                                           in1=grp[h][0:128, 0:NHALF])
                  xsb2.append(x2)

              # stage directly into the A2A input: partition p = 32*i2 + 4*j + ocl
              for ot in range(2):
                  nc.sync.dma_start(
                      out=a2a_in[:, :, 4 * ot:4 * (ot + 1)].rearrange(
                          "j ocl i b s -> (j ocl) i (b s)"),
                      in_=xsb2[ot][:].rearrange(
                          "(i2 jo) f -> jo i2 f", i2=4))

          # ---------------- Reshard ----------------
          tc.strict_bb_all_engine_barrier()
          if profile:
              nc.sync.dma_start(out=a2a_out[:], in_=a2a_in[:])
          else:
              nc.gpsimd.collective_compute(
                  "AllToAll", mybir.AluOpType.bypass,
                  replica_groups=RGRP, ins=[a2a_in.opt()], outs=[a2a_out.opt()])

          # xT: [32*ocl + i | b, q], rows 8..31 of each strip zero
          xT = routing.tile([128, B, S], F16, tag="xT")
          wr = routing.tile([128, S, CO], F16, tag="wr")
          nc.vector.memset(xT, 0.0)
          nc.vector.memset(wr, 0.0)
          nc.sync.dma_start(
              out=xT[:].rearrange("(ocl ii) (jj b) q -> ocl ii jj b q",
                                  ocl=OCL, jj=NCORES)[:, 0:KCAP],
              in_=a2a_out.rearrange("jj ocl i b s -> ocl i jj b s"))
          nc.sync.dma_start(
              out=wr[:].rearrange("(ocl ii) q co -> ocl ii q co",
                                  ocl=OCL)[:, 0:KCAP],
              in_=wroute_d)

          # ---------------- pred generation ----------------
          # PRED[b, o, c, r] fp16, r = rg*36 + q
          PRED = routing.tile([128, O16, C, RLOC], F16, tag="pred")
          P5 = PRED[:].rearrange("p o c (rg q) -> p o c rg q", rg=OCL)
          with ExitStack() as gen_scope:
              gps = gen_scope.enter_context(
                  tc.tile_pool(name="gpsum", bufs=2, space="PSUM"))
              for q in range(S):
                  pp = gps.tile([128, OCL, 512], dt.float32, tag="pp")
                  for rg in range(OCL):
                      nc.tensor.matmul(pp[0:128, rg, 0:CO],
                                       xT[32 * rg:32 * rg + KCAP, :, q],
                                       wr[32 * rg:32 * rg + KCAP, q, :],
                                       start=True, stop=True,
                                       tile_position=(32 * rg, 0))
                  src = pp[:, :, 0:CO].rearrange("p rg (o c) -> p o c rg", o=O16)
                  dst = P5[:, :, :, :, q]
                  if q % 2 == 0:
                      nc.vector.tensor_copy(out=dst, in_=src)
                  else:
                      nc.scalar.copy(out=dst, in_=src)

          # ---------------- routing iterations ----------------
          tc.strict_bb_all_engine_barrier()
          logits = routing.tile([128, C, RLOC], F16, tag="logits")
          e_t = routing.tile([128, C, RLOC], F16, tag="e")
          T = routing.tile([128, O16, C, RLOC], F16, tag="T")
          dc = routing.tile([128, C, RLOC], F16, tag="dc")
          arpack = routing.tile([128, CO + C], dt.float32, tag="arpack")
          s_sb = routing.tile([128, CO + C], dt.float32, tag="s_sb")
          sqs = routing.tile([128, CO], dt.float32, tag="sqs")
          gt1 = routing.tile([1, CO], dt.float32, tag="gt1")
          gt2 = routing.tile([1, CO], dt.float32, tag="gt2")
          vsb = routing.tile([128, CO], dt.float32, tag="vsb")
          onesb = routing.tile([128, 1], dt.float32, tag="onesb")
          onesr = routing.tile([1, 128], dt.float32, tag="onesr")
          rz = routing.tile([128, C], dt.float32, tag="rz")
          nc.sync.dma_start(out=onesb, in_=ones_d)
          nc.sync.dma_start(out=onesr, in_=onesr_d)

          with ExitStack() as it_scope:
              ips = it_scope.enter_context(
                  tc.tile_pool(name="ipsum", bufs=1, space="PSUM"))
              s0p = ips.tile([128, 512], dt.float32, tag="s0p")
              snb = ips.tile([1, CO], dt.float32, tag="snb")
              gbp = ips.tile([128, CO], dt.float32, tag="gbp")

              for t in range(NITER):
                  if t == 0:
                      # s0 = sum_r pred (uniform routing weights), exact in fp32
                      for q in range(S):
                          nc.tensor.matmul(s0p[0:128, 0:CO],
                                           xT[:, :, q], wr[:, q, :],
                                           start=(q == 0), stop=(q == S - 1))
                      nc.scalar.mul(out=arpack[:, 0:CO],
                                    in_=s0p[0:128, 0:CO], mul=1.0 / R)
                      nc.vector.memset(arpack[:, CO:CO + C], 1.0 / NCORES)
                  else:
                      # e = exp(logits), Z = sum_r e, s_hat = sum_r e * pred
                      nc.scalar.activation(out=e_t, in_=logits[:],
                                           func=mybir.ActivationFunctionType.Exp)
                      nc.vector.tensor_reduce(
                          out=arpack[:, CO:CO + C].rearrange(
                              "p (c u) -> p c u", c=C),
                          in_=e_t[:], axis=mybir.AxisListType.X,
                          op=alu.add)
                      EV = e_t[:].rearrange("p c r -> p (c r)")
                      for o in range(O16):
                          eng = nc.vector if o < GP_MUL_SPLIT else nc.gpsimd
                          eng.tensor_mul(
                              out=T[:, o].rearrange("p c r -> p (c r)"),
                              in0=PRED[:, o].rearrange("p c r -> p (c r)"),
                              in1=EV)
                      # in-place tree reduction over r: 144->72->36->18->9
                      w = RLOC
                      while w > 9:
                          hw_ = w // 2
                          nc.vector.tensor_add(
                              out=T[:, 0:GP_MUL_SPLIT, :, 0:hw_],
                              in0=T[:, 0:GP_MUL_SPLIT, :, 0:hw_],
                              in1=T[:, 0:GP_MUL_SPLIT, :, hw_:w])
                          nc.gpsimd.tensor_add(
                              out=T[:, GP_MUL_SPLIT:O16, :, 0:hw_],
                              in0=T[:, GP_MUL_SPLIT:O16, :, 0:hw_],
                              in1=T[:, GP_MUL_SPLIT:O16, :, hw_:w])
                          w = hw_
                      nc.vector.tensor_reduce(
                          out=arpack[:, 0:CO].rearrange("p (oc u) -> p oc u", u=1),
                          in_=T[:, :, :, 0:9].rearrange("p o c r -> p (o c) r"),
                          axis=mybir.AxisListType.X, op=alu.add)

                  ar_in = dram.tile([128, CO + C], dt.float32, tag="arin")
                  ar_out = dram.tile([128, CO + C], dt.float32, tag="arout")
                  nc.sync.dma_start(out=ar_in[:], in_=arpack[:])
                  if profile:
                      nc.sync.dma_start(out=ar_out[:], in_=ar_in[:])
                  else:
                      nc.gpsimd.collective_compute(
                          "AllReduce", alu.add, replica_groups=RGRP,
                          ins=[ar_in.opt()], outs=[ar_out.opt()])
                  nc.sync.dma_start(out=s_sb, in_=ar_out[:])

                  # s = s_hat / Z
                  nc.vector.reciprocal(out=rz, in_=s_sb[:, CO:CO + C])
                  sv = s_sb[:, 0:CO].rearrange("p (o c) -> p o c", o=O16)
                  nc.vector.tensor_mul(
                      out=sv, in0=sv,
                      in1=rz[:].unsqueeze(1).broadcast_to((128, O16, C)))

                  # v = squash(s) over the (full, core-local) batch axis
                  nc.scalar.square(out=sqs, in_=s_sb[:, 0:CO])
                  nc.tensor.matmul(snb[0:1, 0:CO], onesb[:], sqs[:],
                                   start=True, stop=True)
                  nc.scalar.sqrt(out=gt1, in_=snb[0:1, 0:CO])
                  nc.vector.tensor_scalar_add(out=gt2, in0=snb[0:1, 0:CO],
                                              scalar1=1.0)
                  nc.vector.reciprocal(out=gt2, in_=gt2)
                  nc.vector.tensor_mul(out=gt1, in0=gt1, in1=gt2)
                  nc.tensor.matmul(gbp[0:128, 0:CO], onesr[0:1, :], gt1[0:1, :],
                                   start=True, stop=True)
                  nc.vector.tensor_mul(out=vsb, in0=s_sb[:, 0:CO],
                                       in1=gbp[0:128, 0:CO])

                  if t < NITER - 1:
                      # logits += sum_o pred[:,o,c,:] * v[:,o*C+c]
                      for c in range(C):
                          eng = nc.vector if c < GP_DOT_SPLIT else nc.gpsimd
                          tgt = logits[:, c, :] if t == 0 else dc[:, c, :]
                          eng.tensor_scalar_mul(
                              out=tgt, in0=PRED[:, 0, c, :],
                              scalar1=vsb[:, c:c + 1])
                          for o in range(1, O16):
                              eng.scalar_tensor_tensor(
                                  out=tgt, in0=PRED[:, o, c, :],
                                  scalar=vsb[:, o * C + c:o * C + c + 1],
                                  in1=tgt, op0=alu.mult, op1=alu.add)
                          if t > 0:
                              eng.tensor_add(out=logits[:, c, :],
                                             in0=logits[:, c, :],
                                             in1=dc[:, c, :])

              nc.sync.dma_start(out=vout_d, in_=vsb[:])

    _emit()
    nc.compile()
    return nc


def _host_prep(inputs, conv_w, conv_b, route_weights):
    inputs = np.ascontiguousarray(inputs, dtype=np.float32)
    conv_w = np.ascontiguousarray(conv_w, dtype=np.float32)
    conv_b = np.ascontiguousarray(conv_b, dtype=np.float32)
    route_weights = np.ascontiguousarray(route_weights, dtype=np.float32)

    # conv weights -> [81, 2, 128, 256] (shift, cin_t, cin, outch=32k+oc)
    w = conv_w.reshape(OUTCH, CIN, KH, KH)          # [256 outch, 256 cin, 9, 9]
    w = w.transpose(2, 3, 1, 0).reshape(NSHIFT, 2, 128, OUTCH)
    wconv = _to_f32r(w)
    bias = conv_b.reshape(2, 128)

    # route weights per core: [ocl, i, s, o*C+c] fp16, oc = 4*core + ocl
    rw = route_weights.reshape(C, OCAP, S, KCAP, O16)   # [c, oc, s, i, o]
    wroute = []
    for core in range(NCORES):
        blk = rw[:, 4 * core:4 * core + OCL]            # [c, ocl, s, i, o]
        blk = blk.transpose(1, 3, 2, 4, 0).reshape(OCL, KCAP, S, CO)
        wroute.append(np.ascontiguousarray(blk, dtype=np.float16))

    pidx = np.arange(128)
    sel8 = (pidx[:, None] % 32 == np.arange(32)[None, :]).astype(np.float32)
    sel32 = (np.arange(32)[:, None] == pidx[None, :] % 32).astype(np.float32)

    in_maps = []
    for core in range(NCORES):
        in_maps.append({
            "inp": _to_f32r(inputs[BLOC * core:BLOC * (core + 1)]),
            "wconv": wconv,
            "bias": bias,
            "wroute": wroute[core],
            "sel8": sel8,
            "sel32": sel32,
            "ones": np.ones((128, 1), dtype=np.float32),
            "onesr": np.ones((1, 128), dtype=np.float32),
        })
    return in_maps


def kernel(inputs, conv_w, conv_b, route_weights):
    from concourse.bass_utils import run_bass_kernel_spmd

    if "nc" not in _CACHE:
        _CACHE["nc"] = _build_program()
    nc = _CACHE["nc"]

    in_maps = _host_prep(inputs, conv_w, conv_b, route_weights)
    res = run_bass_kernel_spmd(nc, in_maps, core_ids=list(range(NCORES)))
    v = res.results[0]["vout"]                      # [128, 160] = (b, (o c))
    v = v.reshape(B, O16, C).transpose(0, 2, 1)     # -> [B, C, O16]
    return np.ascontiguousarray(v, dtype=np.float32)
